# revision 11
# baseline (speedup 1.0000x reference)
"""CARAFE kernel for Trainium2 (8 NeuronCores, data+H-spatial sharded).

Full inputs in, full output out. Inside: host pre-shards X into per-core
zero-padded slabs (batch x H-quarter), one identical Bass/Tile program runs
SPMD on 8 cores, host gathers the output slabs.
"""

import numpy as np

import concourse.bass as bass
import concourse.mybir as mybir
import concourse.tile as tile
from concourse import bacc, bass_utils

F32 = mybir.dt.float32
AF = mybir.ActivationFunctionType

SCALE = 2
K_UP = 5
K_ENC = 3
EPS = 1e-5
B, C, H, W = 2, 256, 64, 64
MID = 64
ENC = 100  # (SCALE*K_UP)**2
HQ = H // 4          # 16 input rows per core
RS = HQ + 4          # 20 slab rows (halo 2 each side)
XW = W + 4           # 68 padded width
WTW = W + 8          # 72 padded width for weight rows (phase-2 shifts)

_CACHE = {}


def _channel_perm():
    """New enc-channel order: c' = j*20 + i*4 + py*2 + px.
    Original torch order: c = (i*K_UP + j)*4 + py*2 + px."""
    perm = np.zeros(ENC, dtype=np.int64)
    for j in range(K_UP):
        for i in range(K_UP):
            for py in range(SCALE):
                for px in range(SCALE):
                    newc = ((j * K_UP + i) * SCALE + py) * SCALE + px
                    oldc = ((i * K_UP + j) * SCALE + py) * SCALE + px
                    perm[newc] = oldc
    return perm


def _build_program():
    nc = bacc.Bacc("TRN2", target_bir_lowering=False, debug=False)

    # ---- DRAM tensors (per-core inputs; same names on every core) ----
    xslab = nc.dram_tensor("xslab", [128, 2, RS, XW], F32, kind="ExternalInput").ap()
    wcompT = nc.dram_tensor("wcompT", [2, 128, MID], F32, kind="ExternalInput").ap()
    s1 = nc.dram_tensor("s1", [MID, 1], F32, kind="ExternalInput").ap()
    t1 = nc.dram_tensor("t1", [MID, 1], F32, kind="ExternalInput").ap()
    wencT = nc.dram_tensor("wencT", [MID, 9, ENC], F32, kind="ExternalInput").ap()
    s2 = nc.dram_tensor("s2", [ENC, 1], F32, kind="ExternalInput").ap()
    t2 = nc.dram_tensor("t2", [ENC, 1], F32, kind="ExternalInput").ap()
    sel4T = nc.dram_tensor("sel4T", [ENC, 4], F32, kind="ExternalInput").ap()
    selbT = nc.dram_tensor("selbT", [4, ENC], F32, kind="ExternalInput").ap()
    eye100 = nc.dram_tensor("eye100", [ENC, ENC], F32, kind="ExternalInput").ap()
    out = nc.dram_tensor("out", [128, 2, 2 * HQ, 2 * W], F32, kind="ExternalOutput").ap()

    with tile.TileContext(nc) as tc:
        with (
            tc.tile_pool(name="consts", bufs=1) as consts,
            tc.tile_pool(name="xpool", bufs=1) as xpool,
            tc.tile_pool(name="comp", bufs=1) as comppool,
            tc.tile_pool(name="wt", bufs=1) as wtpool,
            tc.tile_pool(name="acc", bufs=1) as accpool,
            tc.tile_pool(name="work", bufs=3) as work,
            tc.tile_pool(name="psum", bufs=1, space="PSUM") as psum,
            tc.tile_pool(name="psumB", bufs=2, space="PSUM") as psumB,
        ):
            # ---- load constants ----
            wcompT_sb = consts.tile([128, 2, MID], F32)
            nc.sync.dma_start(wcompT_sb[:], wcompT.rearrange("k c m -> c k m"))
            s1_sb = consts.tile([MID, 1], F32)
            nc.sync.dma_start(s1_sb[:], s1)
            t1_sb = consts.tile([MID, 1], F32)
            nc.sync.dma_start(t1_sb[:], t1)
            wencT_sb = consts.tile([MID, 9, ENC], F32)
            nc.sync.dma_start(wencT_sb[:], wencT)
            s2_sb = consts.tile([ENC, 1], F32)
            nc.sync.dma_start(s2_sb[:], s2)
            t2_sb = consts.tile([ENC, 1], F32)
            nc.sync.dma_start(t2_sb[:], t2)
            sel4T_sb = consts.tile([ENC, 4], F32)
            nc.sync.dma_start(sel4T_sb[:], sel4T)
            selbT_sb = consts.tile([4, ENC], F32)
            nc.sync.dma_start(selbT_sb[:], selbT)
            eye_sb = consts.tile([ENC, ENC], F32)
            nc.sync.dma_start(eye_sb[:], eye100)

            # ---- load X slab ----
            x_sb = xpool.tile([128, 2, RS, XW], F32)
            nc.sync.dma_start(x_sb[:], xslab)

            # ---- comp = relu(bn1(conv1x1(X))) on 18 rows (slab rows 1..18) ----
            comp_sb = comppool.tile([MID, RS - 2, XW], F32)
            nc.vector.memset(comp_sb[:], 0.0)
            for rt in range(3):  # 3 tiles x 6 rows
                r0 = rt * 6
                ps = psum.tile([MID, 6 * W], F32, tag="ps1")
                for cb in range(2):
                    nc.tensor.matmul(
                        ps[:],
                        wcompT_sb[:, cb, :],
                        x_sb[:, cb, 1 + r0:1 + r0 + 6, 2:2 + W],
                        start=(cb == 0),
                        stop=(cb == 1),
                    )
                nc.scalar.activation(
                    comp_sb[:, r0:r0 + 6, 2:2 + W], ps[:].rearrange("p (r x) -> p r x", r=6),
                    AF.Relu, bias=t1_sb[:], scale=s1_sb[:],
                )

            # ---- per 8-row tile: enc conv, exp, sums, normalize, reassembly ----
            acc = accpool.tile([128, 2, 4, HQ, W], F32)  # [c, cb, s, Y, X_]

            for half in range(2):
                r0 = half * 8
                # enc conv: 9 accumulating taps -> [100, 8*64]
                pse = psum.tile([ENC, 8 * W], F32, tag="ps2")
                for tap in range(9):
                    dy, dx = tap // 3, tap % 3
                    nc.tensor.matmul(
                        pse[:],
                        wencT_sb[:, tap, :],
                        comp_sb[:, r0 + dy:r0 + dy + 8, dx + 1:dx + 1 + W],
                        start=(tap == 0),
                        stop=(tap == 8),
                    )
                # wt = exp(bn2(enc)) into padded rows [100, 8, WTW]
                wt_sb = wtpool.tile([ENC, HQ, WTW], F32, tag="wt")
                wtn_sb = wtpool.tile([ENC, HQ, WTW], F32, tag="wtn")
                nc.scalar.activation(
                    wt_sb[:, r0:r0 + 8, 4:4 + W], pse[:].rearrange("p (r x) -> p r x", r=8),
                    AF.Exp, bias=t2_sb[:], scale=s2_sb[:],
                )
                # k-group sums -> [4, 512]
                pss = psum.tile([4, 8 * W], F32, tag="ps3")
                nc.tensor.matmul(
                    pss[:], sel4T_sb[:], wt_sb[:, r0:r0 + 8, 4:4 + W], start=True, stop=True,
                )
                rec = work.tile([4, 8 * W], F32, tag="rec")
                nc.vector.reciprocal(rec[:], pss[:])
                # broadcast recip back to 100 channels and normalize
                psr = psum.tile([ENC, 8 * W], F32, tag="ps4")
                nc.tensor.matmul(psr[:], selbT_sb[:], rec[:], start=True, stop=True)
                nc.vector.tensor_tensor(
                    out=wtn_sb[:, r0:r0 + 8, 4:4 + W],
                    in0=wt_sb[:, r0:r0 + 8, 4:4 + W],
                    in1=psr[:].rearrange("p (r x) -> p r x", r=8),
                    op=mybir.AluOpType.mult,
                )

                # ---- reassembly: 25 taps x 4 subpixels x 2 c-blocks ----
                for i in range(K_UP):
                    for j in range(K_UP):
                        for s in range(4):
                            ch = (j * K_UP + i) * 4 + s
                            # broadcast wtn[ch] across 128 partitions:
                            # lhsT = I100[:, ch] (free-broadcast) -> out[m,n] = wtn[ch,n]
                            psb = psumB.tile([128, 8 * W], F32, tag="bc")
                            nc.tensor.matmul(
                                psb[:], eye_sb[:, ch:ch + 1].broadcast_to([ENC, 128]),
                                wtn_sb[:, r0:r0 + 8, 4:4 + W],
                                start=True, stop=True,
                            )
                            for cb in range(2):
                                tmp = work.tile([128, 8 * W], F32, tag="tmp")
                                nc.vector.tensor_tensor(
                                    out=tmp[:],
                                    in0=x_sb[:, cb, r0 + i:r0 + i + 8, j:j + W],
                                    in1=psb[:],
                                    op=mybir.AluOpType.mult,
                                )
                                a = acc[:, cb, s, r0:r0 + 8, :]
                                if i == 0 and j == 0:
                                    nc.vector.tensor_copy(a, tmp[:])
                                else:
                                    nc.vector.tensor_tensor(
                                        out=a, in0=a, in1=tmp[:], op=mybir.AluOpType.add,
                                    )

            # ---- interleave subpixels into output layout and store ----
            out_sb = xpool.tile([128, 2, 2 * HQ, 2 * W], F32)
            out_v = out_sb[:].rearrange(
                "p cb (Y py) (X px) -> p cb py px Y X", py=2, px=2
            )
            for cb in range(2):
                for py in range(2):
                    for px in range(2):
                        s = py * 2 + px
                        nc.vector.tensor_copy(
                            out_v[:, cb, py, px, :, :],
                            acc[:, cb, s, :, :],
                        )
            nc.sync.dma_start(out[:], out_sb[:])

    nc.compile()
    return nc


def _prep_shared_inputs(w_comp, b_comp, g1, be1, m1, v1, w_enc, b_enc, g2, be2, m2, v2):
    perm = _channel_perm()
    w_comp = np.asarray(w_comp, np.float32).reshape(MID, C)
    wcompT = np.ascontiguousarray(w_comp.T.reshape(2, 128, MID))
    s1 = (np.asarray(g1) / np.sqrt(np.asarray(v1) + EPS)).astype(np.float32)
    t1 = (np.asarray(b_comp) * s1 + np.asarray(be1) - np.asarray(m1) * s1).astype(np.float32)

    w_enc_p = np.asarray(w_enc, np.float32)[perm]          # [100, 64, 3, 3]
    wencT = np.ascontiguousarray(
        w_enc_p.transpose(1, 2, 3, 0).reshape(MID, 9, ENC)
    )  # [ic, tap(dy*3+dx), oc']
    s2f = (np.asarray(g2) / np.sqrt(np.asarray(v2) + EPS)).astype(np.float32)[perm]
    t2f = (np.asarray(b_enc) * (np.asarray(g2) / np.sqrt(np.asarray(v2) + EPS))
           + np.asarray(be2) - np.asarray(m2) * np.asarray(g2) / np.sqrt(np.asarray(v2) + EPS)
           ).astype(np.float32)[perm]

    sub = np.arange(ENC) % 4
    sel4T = np.zeros((ENC, 4), np.float32)
    sel4T[np.arange(ENC), sub] = 1.0
    selbT = np.ascontiguousarray(sel4T.T)

    return {
        "wcompT": wcompT,
        "s1": s1.reshape(MID, 1), "t1": t1.reshape(MID, 1),
        "wencT": wencT,
        "s2": s2f.reshape(ENC, 1), "t2": t2f.reshape(ENC, 1),
        "sel4T": sel4T, "selbT": selbT,
        "eye100": np.eye(ENC, dtype=np.float32),
    }


def make_in_maps(X, shared):
    X = np.asarray(X, np.float32)
    in_maps = []
    for core in range(8):
        b, q = divmod(core, 4)
        slab = np.zeros((C, RS, XW), np.float32)
        lo, hi = 16 * q - 2, 16 * q + 18
        slo, shi = max(lo, 0), min(hi, H)
        slab[:, slo - lo:shi - lo, 2:2 + W] = X[b, :, slo:shi, :]
        xs = np.ascontiguousarray(slab.reshape(2, 128, RS, XW).transpose(1, 0, 2, 3))
        in_maps.append({"xslab": xs, **shared})
    return in_maps


def kernel(X, w_comp, b_comp, bn1_gamma, bn1_beta, bn1_mean, bn1_var,
           w_enc, b_enc, bn2_gamma, bn2_beta, bn2_mean, bn2_var):
    if "nc" not in _CACHE:
        _CACHE["nc"] = _build_program()
    nc = _CACHE["nc"]

    shared = _prep_shared_inputs(w_comp, b_comp, bn1_gamma, bn1_beta, bn1_mean,
                                 bn1_var, w_enc, b_enc, bn2_gamma, bn2_beta,
                                 bn2_mean, bn2_var)
    in_maps = make_in_maps(X, shared)
    res = bass_utils.run_bass_kernel_spmd(nc, in_maps, core_ids=list(range(8)))

    out = np.zeros((B, C, 2 * H, 2 * W), np.float32)
    for core in range(8):
        b, q = divmod(core, 4)
        o = res.results[core]["out"]  # [128, 2, 32, 128]
        out[b, :, 32 * q:32 * q + 32, :] = o.transpose(1, 0, 2, 3).reshape(C, 32, 128)
    return out


# revision 20
# speedup vs baseline: 4.8274x; 4.8274x over previous
"""CARAFE kernel for Trainium2 (8 NeuronCores, data+H-spatial sharded).

Full inputs in, full output out. Inside: host pre-shards X into per-core
zero-padded slabs (batch x H-quarter), one identical Bass/Tile program runs
SPMD on 8 cores, host gathers the output slabs.
"""

import numpy as np

import concourse.bass as bass
import concourse.mybir as mybir
import concourse.tile as tile
from concourse import bacc, bass_utils

F32 = mybir.dt.float32
F16 = mybir.dt.float16
I16 = mybir.dt.int16
AF = mybir.ActivationFunctionType

SCALE = 2
K_UP = 5
K_ENC = 3
EPS = 1e-5
B, C, H, W = 2, 256, 64, 64
MID = 64
ENC = 100  # (SCALE*K_UP)**2
HQ = H // 4          # 16 input rows per core
RS = HQ + 4          # 20 slab rows (halo 2 each side)
XW = W + 4           # 68 padded width
WTW = W + 8          # 72 padded width for weight rows (phase-2 shifts)

_CACHE = {}


def _channel_perm():
    """New enc-channel order: c' = j*20 + i*4 + py*2 + px.
    Original torch order: c = (i*K_UP + j)*4 + py*2 + px."""
    perm = np.zeros(ENC, dtype=np.int64)
    for j in range(K_UP):
        for i in range(K_UP):
            for py in range(SCALE):
                for px in range(SCALE):
                    newc = ((j * K_UP + i) * SCALE + py) * SCALE + px
                    oldc = ((i * K_UP + j) * SCALE + py) * SCALE + px
                    perm[newc] = oldc
    return perm


def _build_program():
    nc = bacc.Bacc("TRN2", target_bir_lowering=False, debug=False)

    # ---- DRAM tensors (per-core inputs; same names on every core) ----
    xslab = nc.dram_tensor("xslab", [128, 2, RS, XW], F32, kind="ExternalInput").ap()
    wcompT = nc.dram_tensor("wcompT", [2, 128, MID], F32, kind="ExternalInput").ap()
    s1 = nc.dram_tensor("s1", [MID, 1], F32, kind="ExternalInput").ap()
    t1 = nc.dram_tensor("t1", [MID, 1], F32, kind="ExternalInput").ap()
    wencT = nc.dram_tensor("wencT", [MID, 9, ENC], F32, kind="ExternalInput").ap()
    s2 = nc.dram_tensor("s2", [ENC, 1], F32, kind="ExternalInput").ap()
    t2 = nc.dram_tensor("t2", [ENC, 1], F32, kind="ExternalInput").ap()
    sel4T = nc.dram_tensor("sel4T", [ENC, 4], F32, kind="ExternalInput").ap()
    selbT = nc.dram_tensor("selbT", [4, ENC], F32, kind="ExternalInput").ap()
    eye100 = nc.dram_tensor("eye100", [ENC, ENC], F32, kind="ExternalInput").ap()
    out = nc.dram_tensor("out", [128, 2, 2 * HQ, 2 * W], F32, kind="ExternalOutput").ap()

    with tile.TileContext(nc) as tc:
        with (
            tc.tile_pool(name="consts", bufs=1) as consts,
            tc.tile_pool(name="xpool", bufs=1) as xpool,
            tc.tile_pool(name="comp", bufs=1) as comppool,
            tc.tile_pool(name="wt", bufs=1) as wtpool,
            tc.tile_pool(name="acc", bufs=1) as accpool,
            tc.tile_pool(name="work", bufs=3) as work,
            tc.tile_pool(name="psum", bufs=1, space="PSUM") as psum,
            tc.tile_pool(name="psumB", bufs=2, space="PSUM") as psumB,
        ):
            # ---- load constants ----
            wcompT_sb = consts.tile([128, 2, MID], F32)
            nc.sync.dma_start(wcompT_sb[:], wcompT.rearrange("k c m -> c k m"))
            s1_sb = consts.tile([MID, 1], F32)
            nc.sync.dma_start(s1_sb[:], s1)
            t1_sb = consts.tile([MID, 1], F32)
            nc.sync.dma_start(t1_sb[:], t1)
            wencT_sb = consts.tile([MID, 9, ENC], F32)
            nc.sync.dma_start(wencT_sb[:], wencT)
            s2_sb = consts.tile([ENC, 1], F32)
            nc.sync.dma_start(s2_sb[:], s2)
            t2_sb = consts.tile([ENC, 1], F32)
            nc.sync.dma_start(t2_sb[:], t2)
            sel4T_sb = consts.tile([ENC, 4], F32)
            nc.sync.dma_start(sel4T_sb[:], sel4T)
            selbT_sb = consts.tile([4, ENC], F32)
            nc.sync.dma_start(selbT_sb[:], selbT)
            eye_sb = consts.tile([ENC, ENC], F32)
            nc.sync.dma_start(eye_sb[:], eye100)

            # ---- load X slab ----
            x_sb = xpool.tile([128, 2, RS, XW], F32)
            nc.sync.dma_start(x_sb[:], xslab)

            # ---- comp = relu(bn1(conv1x1(X))) on 18 rows (slab rows 1..18) ----
            comp_sb = comppool.tile([MID, RS - 2, XW], F32)
            nc.vector.memset(comp_sb[:], 0.0)
            for rt in range(3):  # 3 tiles x 6 rows
                r0 = rt * 6
                ps = psum.tile([MID, 6 * W], F32, tag="ps1")
                for cb in range(2):
                    nc.tensor.matmul(
                        ps[:],
                        wcompT_sb[:, cb, :],
                        x_sb[:, cb, 1 + r0:1 + r0 + 6, 2:2 + W],
                        start=(cb == 0),
                        stop=(cb == 1),
                    )
                nc.scalar.activation(
                    comp_sb[:, r0:r0 + 6, 2:2 + W], ps[:].rearrange("p (r x) -> p r x", r=6),
                    AF.Relu, bias=t1_sb[:], scale=s1_sb[:],
                )

            # ---- per 8-row tile: enc conv, exp, sums, normalize, reassembly ----
            acc = accpool.tile([128, 2, 4, HQ, W], F32)  # [c, cb, s, Y, X_]

            for half in range(2):
                r0 = half * 8
                # enc conv: 9 accumulating taps -> [100, 8*64]
                pse = psum.tile([ENC, 8 * W], F32, tag="ps2")
                for tap in range(9):
                    dy, dx = tap // 3, tap % 3
                    nc.tensor.matmul(
                        pse[:],
                        wencT_sb[:, tap, :],
                        comp_sb[:, r0 + dy:r0 + dy + 8, dx + 1:dx + 1 + W],
                        start=(tap == 0),
                        stop=(tap == 8),
                    )
                # wt = exp(bn2(enc)) into padded rows [100, 8, WTW]
                wt_sb = wtpool.tile([ENC, HQ, WTW], F32, tag="wt")
                wtn_sb = wtpool.tile([ENC, HQ, WTW], F32, tag="wtn")
                nc.scalar.activation(
                    wt_sb[:, r0:r0 + 8, 4:4 + W], pse[:].rearrange("p (r x) -> p r x", r=8),
                    AF.Exp, bias=t2_sb[:], scale=s2_sb[:],
                )
                # k-group sums -> [4, 512]
                pss = psum.tile([4, 8 * W], F32, tag="ps3")
                nc.tensor.matmul(
                    pss[:], sel4T_sb[:], wt_sb[:, r0:r0 + 8, 4:4 + W], start=True, stop=True,
                )
                rec = work.tile([4, 8 * W], F32, tag="rec")
                nc.vector.reciprocal(rec[:], pss[:])
                # broadcast recip back to 100 channels and normalize
                psr = psum.tile([ENC, 8 * W], F32, tag="ps4")
                nc.tensor.matmul(psr[:], selbT_sb[:], rec[:], start=True, stop=True)
                nc.vector.tensor_tensor(
                    out=wtn_sb[:, r0:r0 + 8, 4:4 + W],
                    in0=wt_sb[:, r0:r0 + 8, 4:4 + W],
                    in1=psr[:].rearrange("p (r x) -> p r x", r=8),
                    op=mybir.AluOpType.mult,
                )

                # ---- reassembly: 25 taps x 4 subpixels x 2 c-blocks ----
                for i in range(K_UP):
                    for j in range(K_UP):
                        for s in range(4):
                            ch = (j * K_UP + i) * 4 + s
                            # broadcast wtn[ch] across 128 partitions:
                            # lhsT = I100[:, ch] (free-broadcast) -> out[m,n] = wtn[ch,n]
                            psb = psumB.tile([128, 8 * W], F32, tag="bc")
                            nc.tensor.matmul(
                                psb[:], eye_sb[:, ch:ch + 1].broadcast_to([ENC, 128]),
                                wtn_sb[:, r0:r0 + 8, 4:4 + W],
                                start=True, stop=True,
                            )
                            for cb in range(2):
                                tmp = work.tile([128, 8 * W], F32, tag="tmp")
                                nc.vector.tensor_tensor(
                                    out=tmp[:],
                                    in0=x_sb[:, cb, r0 + i:r0 + i + 8, j:j + W],
                                    in1=psb[:],
                                    op=mybir.AluOpType.mult,
                                )
                                a = acc[:, cb, s, r0:r0 + 8, :]
                                if i == 0 and j == 0:
                                    nc.vector.tensor_copy(a, tmp[:])
                                else:
                                    nc.vector.tensor_tensor(
                                        out=a, in0=a, in1=tmp[:], op=mybir.AluOpType.add,
                                    )

            # ---- interleave subpixels into output layout and store ----
            out_sb = xpool.tile([128, 2, 2 * HQ, 2 * W], F32)
            out_v = out_sb[:].rearrange(
                "p cb (Y py) (X px) -> p cb py px Y X", py=2, px=2
            )
            for cb in range(2):
                for py in range(2):
                    for px in range(2):
                        s = py * 2 + px
                        nc.vector.tensor_copy(
                            out_v[:, cb, py, px, :, :],
                            acc[:, cb, s, :, :],
                        )
            nc.sync.dma_start(out[:], out_sb[:])

    nc.compile()
    return nc


def _build_program_v2():
    """Banded-matmul reassembly. Per output row-pair Y: 5 shifted PE
    transposes of the normalized softmax weights produce
    V[x'-partition, (j,i,py,px)] (each transpose uses an identity SLICE so
    only its own 20-channel j-group is emitted); a gpsimd local_scatter
    places V into a sparse band matrix band[x', i*256 + py*128 + 2*(x'-j)
    + px]; then 5 fp16 matmuls against the host-pre-transposed X slab
    contract x' and accumulate over the i taps in PSUM."""
    nc = bacc.Bacc("TRN2", target_bir_lowering=False, debug=False)

    xslab = nc.dram_tensor("xslab", [128, 2, RS, XW], F16, kind="ExternalInput").ap()
    xslabT = nc.dram_tensor("xslabT", [XW, 2, RS, 128], F16, kind="ExternalInput").ap()
    wcompT = nc.dram_tensor("wcompT", [2, 128, MID], F16, kind="ExternalInput").ap()
    s1 = nc.dram_tensor("s1", [MID, 1], F32, kind="ExternalInput").ap()
    t1 = nc.dram_tensor("t1", [MID, 1], F32, kind="ExternalInput").ap()
    wencT = nc.dram_tensor("wencT", [MID, 9, ENC], F16, kind="ExternalInput").ap()
    s2 = nc.dram_tensor("s2", [ENC, 1], F32, kind="ExternalInput").ap()
    t2 = nc.dram_tensor("t2", [ENC, 1], F32, kind="ExternalInput").ap()
    sel4T = nc.dram_tensor("sel4T", [ENC, 4], F32, kind="ExternalInput").ap()
    selbT = nc.dram_tensor("selbT", [4, ENC], F32, kind="ExternalInput").ap()
    eye128 = nc.dram_tensor("eye128", [128, 128], F16, kind="ExternalInput").ap()
    scatidx = nc.dram_tensor("scatidx", [80, ENC], I16, kind="ExternalInput").ap()
    out = nc.dram_tensor("out", [128, 2, 2 * HQ, 2 * W], F32, kind="ExternalOutput").ap()

    with tile.TileContext(nc) as tc:
        with (
            tc.tile_pool(name="consts", bufs=1) as consts,
            tc.tile_pool(name="xpool", bufs=1) as xpool,
            tc.tile_pool(name="comp", bufs=1) as comppool,
            tc.tile_pool(name="wt", bufs=1) as wtpool,
            tc.tile_pool(name="vsb", bufs=2) as vpool,
            tc.tile_pool(name="band", bufs=6) as bandpool,
            tc.tile_pool(name="work", bufs=2) as work,
            tc.tile_pool(name="psout", bufs=5, space="PSUM") as psout,
            tc.tile_pool(name="psmisc", bufs=2, space="PSUM") as psmisc,
            tc.tile_pool(name="pstr", bufs=1, space="PSUM") as pstr,
        ):
            wcompT_sb = consts.tile([128, 2, MID], F16)
            nc.sync.dma_start(wcompT_sb[:], wcompT.rearrange("k c m -> c k m"))
            s1_sb = consts.tile([MID, 1], F32)
            nc.sync.dma_start(s1_sb[:], s1)
            t1_sb = consts.tile([MID, 1], F32)
            nc.sync.dma_start(t1_sb[:], t1)
            wencT_sb = consts.tile([MID, 9, ENC], F16)
            nc.sync.dma_start(wencT_sb[:], wencT)
            s2_sb = consts.tile([ENC, 1], F32)
            nc.sync.dma_start(s2_sb[:], s2)
            t2_sb = consts.tile([ENC, 1], F32)
            nc.sync.dma_start(t2_sb[:], t2)
            sel4T_sb = consts.tile([ENC, 4], F32)
            nc.sync.dma_start(sel4T_sb[:], sel4T)
            selbT_sb = consts.tile([4, ENC], F32)
            nc.sync.dma_start(selbT_sb[:], selbT)
            eye_sb = consts.tile([128, 128], F16)
            nc.sync.dma_start(eye_sb[:], eye128)
            sidx_sb = consts.tile([80, ENC], I16)
            nc.sync.dma_start(sidx_sb[:], scatidx)

            x_sb = xpool.tile([128, 2, RS, XW], F16)
            nc.sync.dma_start(x_sb[:], xslab)
            xT_sb = xpool.tile([XW, 2, RS, 128], F16)
            nc.sync.dma_start(xT_sb[:], xslabT)
            out_sb = xpool.tile([128, 2, 2 * HQ, 2 * W], F32)

            # ---- conv1 + bn1 + relu -> comp [64, 18, 68] fp16 ----
            comp_sb = comppool.tile([MID, RS - 2, XW], F16)
            nc.vector.memset(comp_sb[:], 0.0)
            for rt in range(3):
                r0 = rt * 6
                ps = psmisc.tile([128, 512], F32, tag="ps")
                for cb in range(2):
                    nc.tensor.matmul(
                        ps[:MID, :6 * W],
                        wcompT_sb[:, cb, :],
                        x_sb[:, cb, 1 + r0:1 + r0 + 6, 2:2 + W],
                        start=(cb == 0),
                        stop=(cb == 1),
                    )
                nc.scalar.activation(
                    comp_sb[:, r0:r0 + 6, 2:2 + W],
                    ps[:MID, :6 * W].rearrange("p (r x) -> p r x", r=6),
                    AF.Relu, bias=t1_sb[:], scale=s1_sb[:],
                )

            wt_sb = wtpool.tile([ENC, HQ, WTW], F32)
            wtn_sb = wtpool.tile([ENC, HQ, WTW], F16)
            nc.vector.memset(wtn_sb[:, :, 0:4], 0.0)
            nc.vector.memset(wtn_sb[:, :, 4 + W:WTW], 0.0)
            bands = {}
            psums = {}

            for half in range(2):
                r0 = half * 8
                # conv2 (9 taps, fp16) -> psum f32 [100, 512]
                pse = psmisc.tile([128, 512], F32, tag="ps")
                for tap in range(9):
                    dy, dx = tap // 3, tap % 3
                    nc.tensor.matmul(
                        pse[:ENC, :],
                        wencT_sb[:, tap, :],
                        comp_sb[:, r0 + dy:r0 + dy + 8, dx + 1:dx + 1 + W],
                        start=(tap == 0),
                        stop=(tap == 8),
                    )
                # exp(bn2) -> wt rows [100, 8, 64] at col offset 4
                nc.scalar.activation(
                    wt_sb[:, r0:r0 + 8, 4:4 + W],
                    pse[:ENC, :].rearrange("p (r x) -> p r x", r=8),
                    AF.Exp, bias=t2_sb[:], scale=s2_sb[:],
                )
                # k-group sums [4, 512]; reciprocal = exp(-ln(x)) on ACT
                pss = psmisc.tile([128, 512], F32, tag="ps")
                nc.tensor.matmul(
                    pss[:4, :], sel4T_sb[:], wt_sb[:, r0:r0 + 8, 4:4 + W],
                    start=True, stop=True,
                )
                lnt = work.tile([4, 8 * W], F32, tag="ln")
                nc.scalar.activation(lnt[:], pss[:4, :], AF.Ln)
                rec = work.tile([4, 8 * W], F32, tag="rec")
                nc.scalar.activation(rec[:], lnt[:], AF.Exp, scale=-1.0)
                # broadcast recip to 100 channels, normalize -> wtn f32
                psr = psmisc.tile([128, 512], F32, tag="ps")
                nc.tensor.matmul(psr[:ENC, :], selbT_sb[:], rec[:], start=True, stop=True)
                nc.vector.tensor_tensor(
                    out=wtn_sb[:, r0:r0 + 8, 4:4 + W],
                    in0=wt_sb[:, r0:r0 + 8, 4:4 + W],
                    in1=psr[:ENC, :].rearrange("p (r x) -> p r x", r=8),
                    op=mybir.AluOpType.mult,
                )

                # ---- per Y: 5 shifted transposes -> V; local_scatter -> band ----
                for Y in range(r0, r0 + 8):
                    # transpose j writes cols [100j, 100j+100); the valid
                    # j-group (channels 20j..20j+20) thus lands at columns
                    # 120j + t -- a uniform-stride gather view.
                    pv = pstr.tile([XW, 600], F16, tag="pv")
                    for j in range(K_UP):
                        nc.tensor.transpose(
                            pv[:, 100 * j:100 * j + 100],
                            wtn_sb[:, Y, 4 - j:4 - j + XW],
                            eye_sb[:ENC, :ENC],
                        )
                    v_sb = vpool.tile([80, ENC], F16, tag="v")
                    nc.vector.memset(v_sb[64:80, :], 0.0)
                    nc.vector.tensor_copy(
                        v_sb[:XW, :].rearrange("p (j t) -> p j t", j=5),
                        pv[:].rearrange("p (a c) -> p a c", a=5)[:, :, 0:20],
                    )

                    band = bandpool.tile([80, 1280], F16, tag="band")
                    nc.gpsimd.local_scatter(
                        out_ap=band[:],
                        data_ap=v_sb[:],
                        idxs_ap=sidx_sb[:],
                        channels=80,
                        num_elems=1280,
                        num_idxs=ENC,
                    )
                    bands[Y] = band

                # ---- banded matmuls over this half's X rows ----
                srange = range(0, 8) if half == 0 else range(8, RS)
                for s in srange:
                    for i in range(K_UP):
                        Y = s - i
                        if not (0 <= Y < HQ):
                            continue
                        if i == 0:
                            po_t = psout.tile([128, 512], F32, tag="po", name=f"po{Y}")
                            psums[Y] = po_t
                        po = psums[Y]
                        for cb in range(2):
                            nc.tensor.matmul(
                                po[:, 256 * cb:256 * cb + 256],
                                xT_sb[:, cb, s, :],
                                bands[Y][:XW, 256 * i:256 * i + 256],
                                start=(i == 0 and cb == 0),
                                stop=(i == 4 and cb == 1),
                            )
                        if i == 4:
                            nc.vector.tensor_copy(
                                out_sb[:, 0, 2 * Y:2 * Y + 2, :],
                                po[:, 0:256].rearrange("p (py x) -> p py x", py=2),
                            )
                            nc.scalar.copy(
                                out_sb[:, 1, 2 * Y:2 * Y + 2, :],
                                po[:, 256:512].rearrange("p (py x) -> p py x", py=2),
                            )

            nc.sync.dma_start(out[:], out_sb[:])

    nc.compile()
    return nc


def _prep_shared_inputs(w_comp, b_comp, g1, be1, m1, v1, w_enc, b_enc, g2, be2, m2, v2):
    perm = _channel_perm()
    w_comp = np.asarray(w_comp, np.float32).reshape(MID, C)
    wcompT = np.ascontiguousarray(w_comp.T.reshape(2, 128, MID))
    s1 = (np.asarray(g1) / np.sqrt(np.asarray(v1) + EPS)).astype(np.float32)
    t1 = (np.asarray(b_comp) * s1 + np.asarray(be1) - np.asarray(m1) * s1).astype(np.float32)

    w_enc_p = np.asarray(w_enc, np.float32)[perm]          # [100, 64, 3, 3]
    wencT = np.ascontiguousarray(
        w_enc_p.transpose(1, 2, 3, 0).reshape(MID, 9, ENC)
    )  # [ic, tap(dy*3+dx), oc']
    s2f = (np.asarray(g2) / np.sqrt(np.asarray(v2) + EPS)).astype(np.float32)[perm]
    t2f = (np.asarray(b_enc) * (np.asarray(g2) / np.sqrt(np.asarray(v2) + EPS))
           + np.asarray(be2) - np.asarray(m2) * np.asarray(g2) / np.sqrt(np.asarray(v2) + EPS)
           ).astype(np.float32)[perm]

    sub = np.arange(ENC) % 4
    sel4T = np.zeros((ENC, 4), np.float32)
    sel4T[np.arange(ENC), sub] = 1.0
    selbT = np.ascontiguousarray(sel4T.T)

    return {
        "wcompT": wcompT,
        "s1": s1.reshape(MID, 1), "t1": t1.reshape(MID, 1),
        "wencT": wencT,
        "s2": s2f.reshape(ENC, 1), "t2": t2f.reshape(ENC, 1),
        "sel4T": sel4T, "selbT": selbT,
        "eye100": np.eye(ENC, dtype=np.float32),
    }


def _scatter_idx():
    """idx[x', j*20 + i*4 + py*2 + px] = i*256 + py*128 + 2*(x'-j) + px,
    or -1 when x' >= 68 or (x'-j) outside [0, W)."""
    idx = np.full((80, ENC), -1, np.int16)
    for xp in range(XW):
        for j in range(K_UP):
            X_ = xp - j
            if not (0 <= X_ < W):
                continue
            for i in range(K_UP):
                for py in range(2):
                    for px in range(2):
                        col = j * 20 + i * 4 + py * 2 + px
                        idx[xp, col] = i * 256 + py * 128 + 2 * X_ + px
    return idx


def _prep_shared_v2(shared):
    s = dict(shared)
    s["wcompT"] = shared["wcompT"].astype(np.float16)
    s["wencT"] = shared["wencT"].astype(np.float16)
    del s["eye100"]
    s["eye128"] = np.eye(128, dtype=np.float16)
    s["scatidx"] = _scatter_idx()
    return s


def make_in_maps(X, shared, v2=False):
    X = np.asarray(X, np.float32)
    in_maps = []
    for core in range(8):
        b, q = divmod(core, 4)
        slab = np.zeros((C, RS, XW), np.float32)
        lo, hi = 16 * q - 2, 16 * q + 18
        slo, shi = max(lo, 0), min(hi, H)
        slab[:, slo - lo:shi - lo, 2:2 + W] = X[b, :, slo:shi, :]
        xs = np.ascontiguousarray(slab.reshape(2, 128, RS, XW).transpose(1, 0, 2, 3))
        if v2:
            xs16 = xs.astype(np.float16)
            # [x', cb, r, c] transposed slab
            xsT = np.ascontiguousarray(xs16.transpose(3, 1, 2, 0))
            in_maps.append({"xslab": xs16, "xslabT": xsT, **shared})
        else:
            in_maps.append({"xslab": xs, **shared})
    return in_maps


VERSION = 2


def kernel(X, w_comp, b_comp, bn1_gamma, bn1_beta, bn1_mean, bn1_var,
           w_enc, b_enc, bn2_gamma, bn2_beta, bn2_mean, bn2_var):
    key = ("nc", VERSION)
    if key not in _CACHE:
        _CACHE[key] = _build_program_v2() if VERSION == 2 else _build_program()
    nc = _CACHE[key]

    shared = _prep_shared_inputs(w_comp, b_comp, bn1_gamma, bn1_beta, bn1_mean,
                                 bn1_var, w_enc, b_enc, bn2_gamma, bn2_beta,
                                 bn2_mean, bn2_var)
    if VERSION == 2:
        shared = _prep_shared_v2(shared)
    in_maps = make_in_maps(X, shared, v2=(VERSION == 2))
    res = bass_utils.run_bass_kernel_spmd(nc, in_maps, core_ids=list(range(8)))

    out = np.zeros((B, C, 2 * H, 2 * W), np.float32)
    for core in range(8):
        b, q = divmod(core, 4)
        o = res.results[core]["out"]  # [128, 2, 32, 128]
        out[b, :, 32 * q:32 * q + 32, :] = o.transpose(1, 0, 2, 3).reshape(C, 32, 128)
    return out
